# revision 24
# baseline (speedup 1.0000x reference)
"""Trainium2 Bass kernel for the NeuromorphicPrivacyNetwork problem.

Strategy (data-parallel over batch, 8 cores x 32 samples):
  * Everything on-chip lives in transposed layout [neuron, batch] so each
    layer's binary spike tensor feeds the next layer's matmul directly as
    the PE's streaming operand (no transposes anywhere).
  * Layer-0 sees the same `inputs` every step, so inputs @ w0.T is computed
    once on device (true fp32 matmul) and reused for all 16 steps.
  * Refractory period 2.0 with integer step times means "refractory" ==
    "spiked last step"; with thresholds provably positive (min 0.458 for
    this distribution) a refractory neuron can never re-fire, so the update
    collapses to  Vn = (1-sp_prev) * (0.95*V + cur),  sp = Vn > thr.
  * Per-step matmuls use weights split as w = bf16(w) + fp16(w - bf16(w));
    products against {0,1} spikes are exact in the PE (e10m23 accumulate),
    giving fp32-quality results in 2 passes (verified: 0 spike flips vs
    the fp32 reference on CPU and on hardware).
  * cn noise is pre-scaled by 0.05*||w_row|| and thresholds pre-formed as
    th + 0.1*tn on the host; both stream per-step as plain fp32 tensors.
  * The trailing-window privacy cost collapses to sum_s mult[s] * spikes,
    mult[s]=min(10,16-s); the kernel only emits per-(partition,tile,step)
    spike counts, the tiny final weighting happens on the host.
"""

import numpy as np
import ml_dtypes

import concourse.bass as bass
import concourse.bacc as bacc
import concourse.mybir as mybir
import concourse.tile as tile
from concourse import bass_utils

T = 16
B = 256
NCORES = 8
BL = B // NCORES  # 32 batch per core
NT = [16, 8, 4]   # 128-neuron tiles per layer (2048, 1024, 512)
NTT = sum(NT)     # 28 tile slots per step
F32 = mybir.dt.float32
BF16 = mybir.dt.bfloat16
F16 = mybir.dt.float16
ALU = mybir.AluOpType

_compiled = None
last_result = None


def _build(opts=()):
    nc = bacc.Bacc("TRN2", target_bir_lowering=False, debug=False,
                   num_devices=NCORES)

    # ---- DRAM I/O ----
    xT = nc.dram_tensor("xT", [128, 8, BL], F32, kind="ExternalInput")
    w0t = nc.dram_tensor("w0t", [128, 8, 2048], F32, kind="ExternalInput")
    w1h = nc.dram_tensor("w1h", [128, 16, 1024], BF16, kind="ExternalInput")
    w1l = nc.dram_tensor("w1l", [128, 16, 1024], F16, kind="ExternalInput")
    w2h = nc.dram_tensor("w2h", [128, 8, 512], BF16, kind="ExternalInput")
    w2l = nc.dram_tensor("w2l", [128, 8, 512], F16, kind="ExternalInput")
    cn_d = [nc.dram_tensor(f"cn{i}", [T, 128, NT[i], BL], F32,
                           kind="ExternalInput") for i in range(3)]
    thr_d = [nc.dram_tensor(f"thr{i}", [T, 128, NT[i], BL], F32,
                            kind="ExternalInput") for i in range(3)]
    rs_out = nc.dram_tensor("rs_out", [128, T * NTT], F32, kind="ExternalOutput")
    sp_out = nc.dram_tensor("sp_out", [128, NT[2], BL], BF16, kind="ExternalOutput")

    with tile.TileContext(nc) as tc:
        with (
            tc.tile_pool(name="wp", bufs=1) as wp,
            tc.tile_pool(name="state", bufs=2) as st,
            tc.tile_pool(name="work", bufs=2) as wk,
            tc.tile_pool(name="sp", bufs=2) as spp,
            tc.tile_pool(name="noise", bufs=3) as npool,
            tc.tile_pool(name="aux", bufs=1) as aux,
            tc.tile_pool(name="psum", bufs=2, space=bass.MemorySpace.PSUM) as pp,
        ):
            # ---- resident weights / constants ----
            xT_s = wp.tile([128, 8, BL], F32, tag="xT")
            w0_s = wp.tile([128, 8, 2048], F32, tag="w0")
            w1h_s = wp.tile([128, 16, 1024], BF16, tag="w1h")
            w1l_s = wp.tile([128, 16, 1024], F16, tag="w1l")
            w2h_s = wp.tile([128, 8, 512], BF16, tag="w2h")
            w2l_s = wp.tile([128, 8, 512], F16, tag="w2l")

            nc.sync.dma_start(xT_s[:], xT[:])
            for k in range(8):
                nc.sync.dma_start(w0_s[:, k, :], w0t[:, k, :])

            # ---- phase 0: cur0 = (inputs @ w0.T).T, true fp32 ----
            ps0 = pp.tile([128, 16, BL], F32, tag="ps0")
            for j in range(16):
                for k in range(8):
                    nc.tensor.matmul(ps0[:, j, :],
                                     w0_s[:, k, j * 128:(j + 1) * 128],
                                     xT_s[:, k, :],
                                     start=(k == 0), stop=(k == 7))
            cur0 = aux.tile([128, 16, BL], F32, tag="cur0")
            nc.vector.tensor_copy(cur0[:], ps0[:])

            # step-0 noise prefetch, then per-step weights
            def load_noise(s):
                tiles = []
                for i in range(3):
                    c_t = npool.tile([128, NT[i], BL], F32, tag=f"cn{i}")
                    nc.sync.dma_start(c_t[:], cn_d[i][s])
                    t_t = npool.tile([128, NT[i], BL], F32, tag=f"thr{i}")
                    nc.sync.dma_start(t_t[:], thr_d[i][s])
                    tiles.append((c_t, t_t))
                return tiles

            noise = load_noise(0)

            for k in range(16):
                nc.sync.dma_start(w1h_s[:, k, :], w1h[:, k, :])
                nc.sync.dma_start(w1l_s[:, k, :], w1l[:, k, :])
            for k in range(8):
                nc.sync.dma_start(w2h_s[:, k, :], w2h[:, k, :])
                nc.sync.dma_start(w2l_s[:, k, :], w2l[:, k, :])

            # ---- state init ----
            V = [None] * 3
            NSP = [None] * 3
            for i in range(3):
                V[i] = st.tile([128, NT[i], BL], F32, tag=f"V{i}", name=f"V{i}")
                nc.gpsimd.memset(V[i][:], 0.0)
                NSP[i] = st.tile([128, NT[i], BL], F32, tag=f"nsp{i}", name=f"nsp{i}")
                nc.gpsimd.memset(NSP[i][:], 1.0)

            rs = aux.tile([128, T * NTT], F32, tag="rs")

            SP = [None] * 3

            def layer_ew(i, s, cur_src, noise_s):
                """Elementwise LIF update for layer i at step s.
                cur_src: SBUF tile (layer 0) or PSUM tile holding matmul out."""
                c_t, t_t = noise_s[i]
                u = wk.tile([128, NT[i], BL], F32, tag=f"u{i}")
                # u = 0.95*V + cur
                nc.vector.scalar_tensor_tensor(u[:], V[i][:], 0.95, cur_src[:],
                                               op0=ALU.mult, op1=ALU.add)
                z = wk.tile([128, NT[i], BL], F32, tag=f"z{i}")
                nc.vector.tensor_tensor(z[:], u[:], c_t[:], ALU.add)
                vn = st.tile([128, NT[i], BL], F32, tag=f"V{i}", name="vn")
                nc.vector.tensor_tensor(vn[:], z[:], NSP[i][:], ALU.mult)
                sp = spp.tile([128, NT[i], BL], BF16, tag=f"sp{i}")
                nc.vector.tensor_tensor(sp[:], vn[:], t_t[:], ALU.is_gt)
                nsp = st.tile([128, NT[i], BL], F32, tag=f"nsp{i}", name="nsp")
                nc.vector.tensor_scalar(nsp[:], sp[:], -1.0, 1.0,
                                        ALU.mult, ALU.add)
                off = s * NTT + (0, 16, 24)[i]
                nc.vector.tensor_reduce(rs[:, off:off + NT[i]], sp[:],
                                        mybir.AxisListType.X, ALU.add)
                V[i] = vn
                NSP[i] = nsp
                SP[i] = sp

            def layer_mm(i, s):
                """Matmuls for layer i (1 or 2): cur_i.T = w_i @ sp_{i-1}.T"""
                kt = NT[i - 1]            # contraction k-tiles
                jt = NT[i]                # output 128-tiles
                wh, wl = (w1h_s, w1l_s) if i == 1 else (w2h_s, w2l_s)
                ps = pp.tile([128, jt, BL], F32, tag=f"ps{i}")
                spin = SP[i - 1]
                last = 2 * kt - 1
                for j in range(jt):
                    idx = 0
                    for wsrc in (wh, wl):
                        for k in range(kt):
                            nc.tensor.matmul(ps[:, j, :],
                                             wsrc[:, k, j * 128:(j + 1) * 128],
                                             spin[:, k, :],
                                             start=(idx == 0), stop=(idx == last))
                            idx += 1
                return ps

            # ---- software-pipelined time loop ----
            layer_ew(0, 0, cur0, noise)
            for s in range(T):
                ps1 = layer_mm(1, s)
                if s < T - 1:
                    noise_next = load_noise(s + 1)
                    layer_ew(0, s + 1, cur0, noise_next)
                layer_ew(1, s, ps1, noise)
                ps2 = layer_mm(2, s)
                layer_ew(2, s, ps2, noise)
                if s == T - 1:
                    nc.sync.dma_start(sp_out[:], SP[2][:])
                if s < T - 1:
                    noise = noise_next

            nc.sync.dma_start(rs_out[:], rs[:])

    nc.compile()
    return nc


def _split_hi_lo(wt):
    hi = wt.astype(ml_dtypes.bfloat16)
    lo = (wt - hi.astype(np.float32)).astype(np.float16)
    return hi, lo


def kernel(inputs, w0, w1, w2, th0, th1, th2, cn0, cn1, cn2,
           tn0, tn1, tn2, time_steps):
    global _compiled, last_result
    inputs = np.asarray(inputs, np.float32)
    ws = [np.asarray(w, np.float32) for w in (w0, w1, w2)]
    ths = [np.asarray(t, np.float32) for t in (th0, th1, th2)]
    cns = [np.asarray(c, np.float32) for c in (cn0, cn1, cn2)]
    tns = [np.asarray(t, np.float32) for t in (tn0, tn1, tn2)]
    assert int(time_steps) == T

    wn = [np.linalg.norm(w, axis=1).astype(np.float32) for w in ws]

    # weight layouts: wT -> [128, ktiles, nout]
    w0t = np.ascontiguousarray(
        ws[0].T.reshape(8, 128, 2048).transpose(1, 0, 2))
    w1t = ws[1].T.reshape(16, 128, 1024).transpose(1, 0, 2)
    w2t = ws[2].T.reshape(8, 128, 512).transpose(1, 0, 2)
    w1h, w1l = _split_hi_lo(np.ascontiguousarray(w1t))
    w2h, w2l = _split_hi_lo(np.ascontiguousarray(w2t))

    # noise streams: [T, B, N] -> scaled/thresholded -> [T, 128, NT, BL] per core
    cn_s = [cns[i] * (np.float32(0.05) * wn[i]) for i in range(3)]
    thr_s = [ths[i] + tns[i] * np.float32(0.1) for i in range(3)]

    def core_slice(arr, c, nt):
        # [T, BL, N] -> [T, 128, nt, BL]
        a = arr[:, c * BL:(c + 1) * BL, :]
        return np.ascontiguousarray(
            a.reshape(T, BL, nt, 128).transpose(0, 3, 2, 1))

    in_maps = []
    for c in range(NCORES):
        m = {
            "xT": np.ascontiguousarray(
                inputs[c * BL:(c + 1) * BL].T.reshape(8, 128, BL)
                .transpose(1, 0, 2)),
            "w0t": w0t, "w1h": w1h, "w1l": w1l, "w2h": w2h, "w2l": w2l,
        }
        for i in range(3):
            m[f"cn{i}"] = core_slice(cn_s[i], c, NT[i])
            m[f"thr{i}"] = core_slice(thr_s[i], c, NT[i])
        in_maps.append(m)

    if _compiled is None:
        _compiled = _build()
    nc = _compiled

    res = bass_utils.run_bass_kernel_spmd(nc, in_maps,
                                          core_ids=list(range(NCORES)))
    results = res.results
    last_result = res

    # ---- gather ----
    out = np.zeros((B, 512), np.float32)
    rs_all = np.zeros((NCORES, 128, T * NTT), np.float64)
    for c in range(NCORES):
        spc = np.asarray(results[c]["sp_out"]).astype(np.float32)
        out[c * BL:(c + 1) * BL] = spc.transpose(2, 1, 0).reshape(BL, 512)
        rs_all[c] = np.asarray(results[c]["rs_out"]).astype(np.float64)

    # cost / entropy from per-(partition, tile, step) spike counts
    rs_all = rs_all.reshape(NCORES, 128, T, NTT)
    wn_map = np.concatenate(
        [wn[i].reshape(NT[i], 128).T for i in range(3)], axis=1)  # [128, NTT]
    mult = np.minimum(10, T - np.arange(T)).astype(np.float64)    # [T]
    per_step_tile = rs_all.sum(axis=0)                            # [128, T, NTT]
    acc_w = np.einsum("pts,ps,t->", per_step_tile, wn_map.astype(np.float64),
                      mult)
    tot = per_step_tile.sum()
    cost = np.float32(np.float32(0.1) * np.float32(acc_w) / np.float32(10.0))
    cnt = T * B * (2048 + 1024 + 512)
    p1 = np.float32(tot) / np.float32(cnt)
    p0 = np.float32(1.0) - p1
    eps = np.float32(1e-12)
    entropy = -(p1 * np.log2(p1 + eps) + p0 * np.log2(p0 + eps))
    return out, cost, np.float32(entropy)


# revision 32
# speedup vs baseline: 1.1884x; 1.1884x over previous
"""Trainium2 Bass kernel for the NeuromorphicPrivacyNetwork problem.

Strategy (data-parallel over batch, 8 cores x 32 samples):
  * Everything on-chip lives in transposed layout [neuron, batch] so each
    layer's binary spike tensor feeds the next layer's matmul directly as
    the PE's streaming operand (no transposes anywhere).
  * Layer-0 sees the same `inputs` every step, so inputs @ w0.T is computed
    once on device (true fp32 matmul) and reused for all 16 steps.
  * Refractory period 2.0 with integer step times means "refractory" ==
    "spiked last step"; with thresholds provably positive (min 0.458 for
    this distribution) a refractory neuron can never re-fire, so the update
    collapses to  Vn = (1-sp_prev) * (0.95*V + cur),  sp = Vn > thr.
  * Per-step matmuls use weights split as w = bf16(w) + fp16(w - bf16(w));
    products against {0,1} spikes are exact in the PE (e10m23 accumulate),
    giving fp32-quality results in 2 passes (verified: 0 spike flips vs
    the fp32 reference on CPU and on hardware).
  * cn noise is pre-scaled by 0.05*||w_row|| and thresholds pre-formed as
    th + 0.1*tn on the host; both stream per-step as plain fp32 tensors.
  * The trailing-window privacy cost collapses to sum_s mult[s] * spikes,
    mult[s]=min(10,16-s); the kernel only emits per-(partition,tile,step)
    spike counts, the tiny final weighting happens on the host.
"""

import numpy as np
import ml_dtypes

import concourse.bass as bass
import concourse.bacc as bacc
import concourse.mybir as mybir
import concourse.tile as tile
from concourse import bass_utils

T = 16
B = 256
NCORES = 8
BL = B // NCORES  # 32 batch per core
NT = [16, 8, 4]   # 128-neuron tiles per layer (2048, 1024, 512)
NTT = sum(NT)     # 28 tile slots per step
F32 = mybir.dt.float32
BF16 = mybir.dt.bfloat16
F16 = mybir.dt.float16
ALU = mybir.AluOpType

_compiled = None
last_result = None


def _build(opts=()):
    nc = bacc.Bacc("TRN2", target_bir_lowering=False, debug=False,
                   num_devices=NCORES)

    # ---- DRAM I/O ----
    xT = nc.dram_tensor("xT", [128, 8, BL], F32, kind="ExternalInput")
    w0t = nc.dram_tensor("w0t", [128, 8, 2048], F32, kind="ExternalInput")
    w1h = nc.dram_tensor("w1h", [128, 16, 1024], BF16, kind="ExternalInput")
    w1l = nc.dram_tensor("w1l", [128, 16, 1024], F16, kind="ExternalInput")
    w2h = nc.dram_tensor("w2h", [128, 8, 512], BF16, kind="ExternalInput")
    w2l = nc.dram_tensor("w2l", [128, 8, 512], F16, kind="ExternalInput")
    cn_d = [nc.dram_tensor(f"cn{i}", [T, 128, NT[i], BL], F32,
                           kind="ExternalInput") for i in range(3)]
    thr_d = [nc.dram_tensor(f"thr{i}", [T, 128, NT[i], BL], F32,
                            kind="ExternalInput") for i in range(3)]
    rs_out = nc.dram_tensor("rs_out", [128, T * NTT + 16], F32, kind="ExternalOutput")
    sp_out = nc.dram_tensor("sp_out", [128, NT[2], BL], BF16, kind="ExternalOutput")

    with tile.TileContext(nc) as tc:
        with (
            tc.tile_pool(name="wp", bufs=1) as wp,
            tc.tile_pool(name="state", bufs=2) as st,
            tc.tile_pool(name="work", bufs=2) as wk,
            tc.tile_pool(name="sp", bufs=2) as spp,
            tc.tile_pool(name="noise", bufs=3) as npool,
            tc.tile_pool(name="aux", bufs=1) as aux,
            tc.tile_pool(name="psum", bufs=2, space=bass.MemorySpace.PSUM) as pp,
        ):
            # ---- resident weights / constants ----
            xT_s = wp.tile([128, 8, BL], F32, tag="xT")
            w0_s = wp.tile([128, 8, 2048], F32, tag="w0")
            w1h_s = wp.tile([128, 16, 1024], BF16, tag="w1h")
            w1l_s = wp.tile([128, 16, 1024], F16, tag="w1l")
            w2h_s = wp.tile([128, 8, 512], BF16, tag="w2h")
            w2l_s = wp.tile([128, 8, 512], F16, tag="w2l")

            nc.sync.dma_start(xT_s[:], xT[:])
            for k in range(8):
                nc.sync.dma_start(w0_s[:, k, :], w0t[:, k, :])

            # ---- phase 0: cur0 = (inputs @ w0.T).T, true fp32 ----
            ps0 = pp.tile([128, 16, BL], F32, tag="ps0")
            for j in range(16):
                for k in range(8):
                    nc.tensor.matmul(ps0[:, j, :],
                                     w0_s[:, k, j * 128:(j + 1) * 128],
                                     xT_s[:, k, :],
                                     start=(k == 0), stop=(k == 7))
            cur0 = aux.tile([128, 16, BL], F32, tag="cur0")
            nc.vector.tensor_copy(cur0[:], ps0[:])

            # step-0 noise prefetch, then per-step weights
            def load_noise(s):
                tiles = []
                for i in range(3):
                    c_t = npool.tile([128, NT[i], BL], F32, tag=f"cn{i}")
                    nc.sync.dma_start(c_t[:], cn_d[i][s])
                    t_t = npool.tile([128, NT[i], BL], F32, tag=f"thr{i}")
                    nc.sync.dma_start(t_t[:], thr_d[i][s])
                    tiles.append((c_t, t_t))
                return tiles

            noise = load_noise(0)

            for k in range(16):
                nc.sync.dma_start(w1h_s[:, k, :], w1h[:, k, :])
                nc.sync.dma_start(w1l_s[:, k, :], w1l[:, k, :])
            for k in range(8):
                nc.sync.dma_start(w2h_s[:, k, :], w2h[:, k, :])
                nc.sync.dma_start(w2l_s[:, k, :], w2l[:, k, :])

            # ---- state init ----
            V = [None] * 3
            NSP = [None] * 3
            for i in range(3):
                V[i] = st.tile([128, NT[i], BL], F32, tag=f"V{i}", name=f"V{i}")
                nc.gpsimd.memset(V[i][:], 0.0)
                NSP[i] = st.tile([128, NT[i], BL], BF16, tag=f"nsp{i}", name=f"nsp{i}")
                nc.gpsimd.memset(NSP[i][:], 1.0)

            rs = aux.tile([128, T * NTT + 16], F32, tag="rs")
            nc.gpsimd.memset(rs[:, T * NTT:], 0.0)

            SP = [None] * 3

            def layer_ew(i, s, cur_src, noise_s):
                """Elementwise LIF update for layer i at step s.
                cur_src: SBUF tile (layer 0) or PSUM tile holding matmul out."""
                c_t, t_t = noise_s[i]
                nspP = NSP[i]
                u = wk.tile([128, NT[i], BL], F32, tag=f"u{i}")
                # u = 0.95*V + cur;  spike test vs preshifted thr2 so the
                # critical chain is u -> spc -> sp; membrane state (z, vn)
                # is maintained off-path on GPSIMD.
                nc.vector.scalar_tensor_tensor(u[:], V[i][:], 0.95, cur_src[:],
                                               op0=ALU.mult, op1=ALU.add)
                spc = wk.tile([128, NT[i], BL], BF16, tag=f"spc{i}")
                nc.vector.tensor_tensor(spc[:], u[:], t_t[:], ALU.is_gt)
                sp = spp.tile([128, NT[i], BL], BF16, tag=f"sp{i}")
                nc.vector.tensor_tensor(sp[:], spc[:], nspP[:], ALU.mult)
                nsp = st.tile([128, NT[i], BL], BF16, tag=f"nsp{i}", name="nsp")
                nc.scalar.activation(nsp[:], sp[:],
                                     mybir.ActivationFunctionType.Copy,
                                     bias=1.0, scale=-1.0)
                z = wk.tile([128, NT[i], BL], F32, tag=f"z{i}")
                nc.gpsimd.tensor_tensor(z[:], u[:], c_t[:], ALU.add)
                vn = st.tile([128, NT[i], BL], F32, tag=f"V{i}", name="vn")
                nc.gpsimd.tensor_tensor(vn[:], z[:], nspP[:], ALU.mult)
                off = s * NTT + (0, 16, 24)[i]
                nc.vector.tensor_reduce(rs[:, off:off + NT[i]], sp[:],
                                        mybir.AxisListType.X, ALU.add)
                V[i] = vn
                NSP[i] = nsp
                SP[i] = sp

            def layer_mm(i, s):
                """Matmuls for layer i (1 or 2): cur_i.T = w_i @ sp_{i-1}.T"""
                kt = NT[i - 1]            # contraction k-tiles
                jt = NT[i]                # output 128-tiles
                wh, wl = (w1h_s, w1l_s) if i == 1 else (w2h_s, w2l_s)
                ps = pp.tile([128, jt, BL], F32, tag=f"ps{i}")
                spin = SP[i - 1]
                last = 2 * kt - 1
                for j in range(jt):
                    idx = 0
                    for wsrc in (wh, wl):
                        for k in range(kt):
                            nc.tensor.matmul(ps[:, j, :],
                                             wsrc[:, k, j * 128:(j + 1) * 128],
                                             spin[:, k, :],
                                             start=(idx == 0), stop=(idx == last))
                            idx += 1
                return ps

            # ---- software-pipelined time loop ----
            layer_ew(0, 0, cur0, noise)
            for s in range(T):
                ps1 = layer_mm(1, s)
                if s < T - 1:
                    noise_next = load_noise(s + 1)
                    layer_ew(0, s + 1, cur0, noise_next)
                layer_ew(1, s, ps1, noise)
                ps2 = layer_mm(2, s)
                layer_ew(2, s, ps2, noise)
                if s == T - 1:
                    nc.sync.dma_start(sp_out[:], SP[2][:])
                if s < T - 1:
                    noise = noise_next

            nc.sync.dma_start(rs_out[:], rs[:])

    nc.compile()
    return nc


def _split_hi_lo(wt):
    hi = wt.astype(ml_dtypes.bfloat16)
    lo = (wt - hi.astype(np.float32)).astype(np.float16)
    return hi, lo


def kernel(inputs, w0, w1, w2, th0, th1, th2, cn0, cn1, cn2,
           tn0, tn1, tn2, time_steps):
    global _compiled, last_result
    inputs = np.asarray(inputs, np.float32)
    ws = [np.asarray(w, np.float32) for w in (w0, w1, w2)]
    ths = [np.asarray(t, np.float32) for t in (th0, th1, th2)]
    cns = [np.asarray(c, np.float32) for c in (cn0, cn1, cn2)]
    tns = [np.asarray(t, np.float32) for t in (tn0, tn1, tn2)]
    assert int(time_steps) == T

    wn = [np.linalg.norm(w, axis=1).astype(np.float32) for w in ws]

    # weight layouts: wT -> [128, ktiles, nout]
    w0t = np.ascontiguousarray(
        ws[0].T.reshape(8, 128, 2048).transpose(1, 0, 2))
    w1t = ws[1].T.reshape(16, 128, 1024).transpose(1, 0, 2)
    w2t = ws[2].T.reshape(8, 128, 512).transpose(1, 0, 2)
    w1h, w1l = _split_hi_lo(np.ascontiguousarray(w1t))
    w2h, w2l = _split_hi_lo(np.ascontiguousarray(w2t))

    # noise streams: [T, B, N] -> scaled/thresholded -> [T, 128, NT, BL] per core
    cn_s = [cns[i] * (np.float32(0.05) * wn[i]) for i in range(3)]
    # preshifted threshold: spike test (0.95V + P) > th + 0.1*tn - cn'
    thr_s = [(ths[i] + tns[i] * np.float32(0.1)) - cn_s[i] for i in range(3)]

    def core_slice(arr, c, nt):
        # [T, BL, N] -> [T, 128, nt, BL]
        a = arr[:, c * BL:(c + 1) * BL, :]
        return np.ascontiguousarray(
            a.reshape(T, BL, nt, 128).transpose(0, 3, 2, 1))

    in_maps = []
    for c in range(NCORES):
        m = {
            "xT": np.ascontiguousarray(
                inputs[c * BL:(c + 1) * BL].T.reshape(8, 128, BL)
                .transpose(1, 0, 2)),
            "w0t": w0t, "w1h": w1h, "w1l": w1l, "w2h": w2h, "w2l": w2l,
        }
        for i in range(3):
            m[f"cn{i}"] = core_slice(cn_s[i], c, NT[i])
            m[f"thr{i}"] = core_slice(thr_s[i], c, NT[i])
        in_maps.append(m)

    if _compiled is None:
        _compiled = _build()
    nc = _compiled

    res = bass_utils.run_bass_kernel_spmd(nc, in_maps,
                                          core_ids=list(range(NCORES)))
    results = res.results
    last_result = res

    # ---- gather ----
    out = np.zeros((B, 512), np.float32)
    rs_all = np.zeros((NCORES, 128, T * NTT), np.float64)
    for c in range(NCORES):
        spc = np.asarray(results[c]["sp_out"]).astype(np.float32)
        out[c * BL:(c + 1) * BL] = spc.transpose(2, 1, 0).reshape(BL, 512)
        rs_all[c] = np.asarray(results[c]["rs_out"])[:, :T * NTT].astype(np.float64)

    # cost / entropy from per-(partition, tile, step) spike counts
    rs_all = rs_all.reshape(NCORES, 128, T, NTT)
    wn_map = np.concatenate(
        [wn[i].reshape(NT[i], 128).T for i in range(3)], axis=1)  # [128, NTT]
    mult = np.minimum(10, T - np.arange(T)).astype(np.float64)    # [T]
    per_step_tile = rs_all.sum(axis=0)                            # [128, T, NTT]
    acc_w = np.einsum("pts,ps,t->", per_step_tile, wn_map.astype(np.float64),
                      mult)
    tot = per_step_tile.sum()
    cost = np.float32(np.float32(0.1) * np.float32(acc_w) / np.float32(10.0))
    cnt = T * B * (2048 + 1024 + 512)
    p1 = np.float32(tot) / np.float32(cnt)
    p0 = np.float32(1.0) - p1
    eps = np.float32(1e-12)
    entropy = -(p1 * np.log2(p1 + eps) + p0 * np.log2(p0 + eps))
    return out, cost, np.float32(entropy)
